# revision 1
# baseline (speedup 1.0000x reference)
"""Trainium2 Bass kernel for nn_Ir_Consistency_Loss (gnn_message_passing).

loss = mean_e (1 - re[src_e].re[dst_e]) * ||ir_h[src_e] - ir_h[dst_e]||^2

Edge-parallel across 8 NeuronCores, dma_gather-based:
  - Host: gather table G = concat(re_, ir_h) [N, 256] f32, split into halves
    GA = G[:25000], GB = G[25000:] so local row ids fit dma_gather's int16.
  - Edges bucketed by (src-half, dst-half) into 4 streams; each bucket is
    sharded over 8 cores and padded (pad edge = local (0,0)) to a common
    per-core tile count; pad contributions are subtracted exactly on host.
  - Device, per tile of 2048 edges: two dma_gathers (2048 rows x 1KB each,
    one SWDGE instruction per endpoint) + DVE:
      agree = sum(u_re*v_re), sq = sum((u_ir-v_ir)^2) per edge,
      partials[:, t] = sum_j (agree-1)*sq   [negated tile loss].
  - Host: loss = -(sum of partials - pad corrections) / E.

dma_gather applies the fixed bijection j -> out[j%128, j//128] with index
SBUF layout j -> [j%16, j//16] replicated on the 8 16-partition groups
(HW-verified). src and dst use the identical layout, so per-edge slots align
and the final sum is permutation-invariant.
"""

import numpy as np

import concourse.bacc as bacc
import concourse.mybir as mybir
import concourse.tile as tile
from concourse.bass_utils import run_bass_kernel_spmd

N_NODES = 50000
HALF = 25000
D = 128
N_CORES = 8
P = 128
SLOTS = 16                 # edges per partition per tile
TILE_E = P * SLOTS         # 2048 edges per tile
IDX_COLS = TILE_E // 16    # int16 idx columns (wrap-16 layout)

_cache = {}


def _build_program(tiles_per_bucket):
    key = tuple(tiles_per_bucket)
    if key in _cache:
        return _cache[key]
    total_tiles = sum(tiles_per_bucket)
    nc = bacc.Bacc("TRN2", target_bir_lowering=False, debug=False,
                   num_devices=N_CORES)
    ga = nc.dram_tensor("ga", [HALF, 2 * D], mybir.dt.float32,
                        kind="ExternalInput")
    gb = nc.dram_tensor("gb", [N_NODES - HALF, 2 * D], mybir.dt.float32,
                        kind="ExternalInput")
    src = nc.dram_tensor("src_idx", [total_tiles * P, IDX_COLS],
                         mybir.dt.int16, kind="ExternalInput")
    dst = nc.dram_tensor("dst_idx", [total_tiles * P, IDX_COLS],
                         mybir.dt.int16, kind="ExternalInput")
    out = nc.dram_tensor("partial", [P, 1], mybir.dt.float32,
                         kind="ExternalOutput")

    fp32 = mybir.dt.float32
    Alu = mybir.AluOpType
    X = mybir.AxisListType.X
    stab = [ga, ga, gb, gb]
    dtab = [ga, gb, ga, gb]

    with tile.TileContext(nc) as tc:
        with (
            tc.tile_pool(name="idx", bufs=3) as ipool,
            tc.tile_pool(name="gath", bufs=3) as gpool,
            tc.tile_pool(name="scr", bufs=2) as spool,
            tc.tile_pool(name="stats", bufs=1) as stpool,
        ):
            partials = stpool.tile([P, total_tiles], fp32, tag="partials")
            t = 0
            for b in range(4):
                for _ in range(tiles_per_bucket[b]):
                    si = ipool.tile([P, IDX_COLS], mybir.dt.int16, tag="si")
                    di = ipool.tile([P, IDX_COLS], mybir.dt.int16, tag="di")
                    nc.sync.dma_start(out=si[:], in_=src[t * P:(t + 1) * P, :])
                    nc.sync.dma_start(out=di[:], in_=dst[t * P:(t + 1) * P, :])

                    u = gpool.tile([P, SLOTS, 2 * D], fp32, tag="u")
                    v = gpool.tile([P, SLOTS, 2 * D], fp32, tag="v")
                    nc.gpsimd.dma_gather(u[:], stab[b][:], si[:], TILE_E,
                                         TILE_E, 2 * D, single_packet=False)
                    nc.gpsimd.dma_gather(v[:], dtab[b][:], di[:], TILE_E,
                                         TILE_E, 2 * D, single_packet=False)

                    prod = spool.tile([P, SLOTS, D], fp32, tag="prod")
                    agree = spool.tile([P, SLOTS], fp32, tag="agree")
                    diff = spool.tile([P, SLOTS, D], fp32, tag="diff")
                    sq = spool.tile([P, SLOTS, D], fp32, tag="sq")
                    sqsum = spool.tile([P, SLOTS], fp32, tag="sqsum")
                    junk = spool.tile([P, SLOTS], fp32, tag="junk")

                    nc.vector.tensor_tensor(out=prod[:], in0=u[:, :, 0:D],
                                            in1=v[:, :, 0:D], op=Alu.mult)
                    nc.vector.tensor_reduce(out=agree[:], in_=prod[:], axis=X,
                                            op=Alu.add)
                    nc.vector.tensor_tensor(out=diff[:], in0=u[:, :, D:2 * D],
                                            in1=v[:, :, D:2 * D],
                                            op=Alu.subtract)
                    nc.vector.tensor_tensor(out=sq[:], in0=diff[:],
                                            in1=diff[:], op=Alu.mult)
                    nc.vector.tensor_reduce(out=sqsum[:], in_=sq[:], axis=X,
                                            op=Alu.add)
                    nc.vector.scalar_tensor_tensor(
                        out=junk[:], in0=agree[:], scalar=1.0, in1=sqsum[:],
                        op0=Alu.subtract, op1=Alu.mult,
                        accum_out=partials[:, t:t + 1])
                    t += 1

            total = stpool.tile([P, 1], fp32, tag="total")
            nc.vector.tensor_reduce(out=total[:], in_=partials[:], axis=X,
                                    op=Alu.add)
            nc.sync.dma_start(out=out[:], in_=total[:])
    nc.compile()
    _cache[key] = nc
    return nc


def _wrap_idx(flat_idx):
    """[n_tiles, TILE_E] local ids -> [n_tiles*P, IDX_COLS] int16 blocks.
    Logical j -> [j % 16, j // 16], replicated on all 8 16-row groups."""
    nt = flat_idx.shape[0]
    j = np.arange(TILE_E)
    w = np.zeros((nt, 16, IDX_COLS), np.int16)
    w[:, j % 16, j // 16] = flat_idx.astype(np.int16)
    return np.ascontiguousarray(np.tile(w, (1, 8, 1))).reshape(nt * P, IDX_COLS)


def kernel(re_, ir_h, src, dst):
    re_ = np.asarray(re_, dtype=np.float32)
    ir_h = np.asarray(ir_h, dtype=np.float32)
    g = np.ascontiguousarray(np.concatenate([re_, ir_h], axis=1))
    ga, gb = np.ascontiguousarray(g[:HALF]), np.ascontiguousarray(g[HALF:])

    s = np.asarray(src).astype(np.int64)
    d = np.asarray(dst).astype(np.int64)
    e_total = s.shape[0]
    bucket = (s >= HALF) * 2 + (d >= HALF)

    src_blocks = [[] for _ in range(N_CORES)]
    dst_blocks = [[] for _ in range(N_CORES)]
    tiles_per_bucket = []
    pad_counts = [0, 0, 0, 0]
    for b in range(4):
        m = bucket == b
        sb = s[m] - (HALF if b >= 2 else 0)
        db = d[m] - (HALF if b % 2 == 1 else 0)
        n = sb.shape[0]
        per_core = -(-n // N_CORES)
        nt = max(1, -(-per_core // TILE_E))
        tiles_per_bucket.append(nt)
        cap = nt * TILE_E
        pad_counts[b] = cap * N_CORES - n
        sp = np.zeros(cap * N_CORES, np.int64)
        dp = np.zeros(cap * N_CORES, np.int64)
        sp[:n] = sb
        dp[:n] = db
        for c in range(N_CORES):
            src_blocks[c].append(sp[c * cap:(c + 1) * cap].reshape(nt, TILE_E))
            dst_blocks[c].append(dp[c * cap:(c + 1) * cap].reshape(nt, TILE_E))

    in_maps = []
    for c in range(N_CORES):
        sw = _wrap_idx(np.concatenate(src_blocks[c], axis=0))
        dw = _wrap_idx(np.concatenate(dst_blocks[c], axis=0))
        in_maps.append({"ga": ga, "gb": gb, "src_idx": sw, "dst_idx": dw})

    nc = _build_program(tuple(tiles_per_bucket))
    res = run_bass_kernel_spmd(nc, in_maps, core_ids=list(range(N_CORES)))
    tot = 0.0
    for r in res.results:
        tot += float(r["partial"].sum(dtype=np.float64))
    loss_sum = -tot

    base = [(0, 0), (0, HALF), (HALF, 0), (HALF, HALF)]
    for b in range(4):
        if pad_counts[b]:
            i0, j0 = base[b]
            agree = float(np.dot(re_[i0].astype(np.float64),
                                 re_[j0].astype(np.float64)))
            sqd = float(((ir_h[i0].astype(np.float64)
                          - ir_h[j0].astype(np.float64)) ** 2).sum())
            loss_sum -= pad_counts[b] * (1.0 - agree) * sqd
    return np.float32(loss_sum / e_total)

